# revision 18
# baseline (speedup 1.0000x reference)
"""AdversarialContrastiveLoss on 8 trn2 NeuronCores.

Strategy (per sharding hint): shard rows of the 8192x8192 similarity matrix
across 8 cores (1024 query rows each); every core holds all 8192 keys.

Host-side prep (index metadata only — all O(B^2 D) work stays on device):
  * rows sorted by affordance id, so each core's queries' positive keys live
    in one contiguous "window" of <= PASS2W columns
  * per-core key rotation puts that window at columns [0, PASS2W); all
    own-affordance keys of every query of the core live inside it
  * projections pre-transposed to [D, B] so the PE gets contraction on
    partitions without on-device transposes

Device (per core, fused single pass over sim):
  * sim tile [128 q x 1024 k] per PSUM buffer via float32r matmuls (N=512)
  * hard-negative row max via tensor_tensor_reduce (max) per PSUM group; the
    neg_mask (aff equal -> exclude) is an additive -2^30 key-aff equality
    mask, only needed over the first two groups
  * margin losses only need sim values inside the window: relu + masked sum
    fused into one ScalarE activation(accum) per tile; +/-4.0 pos-mask
    offsets fused into GpSimd scalar_tensor_tensor ops (small offset keeps
    the f32 sims exact; 4/2^30 is an exact power of two)
  * outputs per-partition loss sums; num_pairs is pure id metadata (host)
"""

import os
import sys

try:
    import concourse  # noqa: F401  (resolves via the container's sitecustomize)
except ImportError:  # pragma: no cover - fallback for bare environments
    for _p in ("/root/.axon_site/_ro/trn_rl_repo", "/opt/trn_rl_repo"):
        if os.path.isdir(_p) and _p not in sys.path:
            sys.path.append(_p)

import numpy as np

import concourse.bass as bass
import concourse.tile as tile
from concourse import bacc, bass_utils, mybir

F32 = mybir.dt.float32
F32R = mybir.dt.float32r  # TF32-like PE mode: 4x matmul throughput vs fp32
ALU = mybir.AluOpType
ACTF = mybir.ActivationFunctionType
MM_DT = F32R

B = 8192
D = 256
NCORES = 8
RPC = B // NCORES            # query rows per core
NT = RPC // 128              # query tiles per core (8)
GW = 1024                    # cols per PSUM group
NGRP = B // GW               # 8 psum groups
MW = 2 * GW                  # width of the masked (own-aff) region
MARGIN = 0.2
BIGNEG = -float(2 ** 30)     # pass-1 max mask (masked values are discarded)
POSC = 4.0                   # pass-2 mask offset (small: keeps sim exact)

_cache = {}


def build_kernel(p2w):
    """p2w: pass-2 window width (cols holding every positive pair)."""
    assert p2w <= MW
    nc = bacc.Bacc("TRN2", target_bir_lowering=False)

    kt = nc.dram_tensor("kt", [D, B], F32, kind="ExternalInput")
    qt = nc.dram_tensor("qt", [D, RPC], F32, kind="ExternalInput")
    affw = nc.dram_tensor("affw", [128, MW], F32, kind="ExternalInput")
    cidw = nc.dram_tensor("cidw", [128, p2w], F32, kind="ExternalInput")
    qaff = nc.dram_tensor("qaff", [128, NT], F32, kind="ExternalInput")
    qcid = nc.dram_tensor("qcid", [128, NT], F32, kind="ExternalInput")
    out = nc.dram_tensor("out", [128, NT], F32, kind="ExternalOutput")

    with tile.TileContext(nc) as tc:
        with tc.tile_pool(name="singles", bufs=1) as singles, \
             tc.tile_pool(name="wk", bufs=8) as wk, \
             tc.tile_pool(name="dmp", bufs=2) as dmp, \
             tc.tile_pool(name="small", bufs=4) as small, \
             tc.tile_pool(name="psum", bufs=4, space="PSUM") as psum:

            qaff_t = singles.tile([128, NT], F32, tag="qaff")
            qcid_t = singles.tile([128, NT], F32, tag="qcid")
            affw_t = singles.tile([128, MW], F32, tag="affw")
            cidw_t = singles.tile([128, p2w], F32, tag="cidw")
            for t, src in [(qaff_t, qaff), (qcid_t, qcid), (affw_t, affw),
                           (cidw_t, cidw)]:
                nc.scalar.dma_start(out=t, in_=src[:, :])

            # queries (every matmul needs them) then keys in group order,
            # spread across issue queues so groups land incrementally
            qtt = [singles.tile([128, RPC], MM_DT, tag=f"qt{k}",
                                name=f"qtt{k}")
                   for k in range(2)]
            for k in range(2):
                nc.sync.dma_start(out=qtt[k],
                                  in_=qt[k * 128:(k + 1) * 128, :]
                                  .bitcast(MM_DT))
            ktt = [[singles.tile([128, GW], MM_DT, tag=f"kt{k}g{g}",
                                 name=f"ktt{k}g{g}")
                    for g in range(NGRP)] for k in range(2)]
            dma_engines = [nc.sync, nc.scalar, nc.gpsimd]
            di = 0
            for g in range(NGRP):
                for k in range(2):
                    dma_engines[di % len(dma_engines)].dma_start(
                        out=ktt[k][g],
                        in_=kt[k * 128:(k + 1) * 128, g * GW:(g + 1) * GW]
                        .bitcast(MM_DT))
                    di += 1

            zeros = singles.tile([128, GW], F32, tag="zeros")
            nc.vector.memset(zeros, 0.0)
            lsum = singles.tile([128, NT], F32, tag="lsum")

            for m in range(NT):
                # e4 = -POSC * (key_aff == query_aff): additive neg-mask for
                # the row max (sims are in [-1,1], so -4 is "minus infinity")
                # AND the pass-2 positive-pair offset, in one tensor.
                e4 = wk.tile([128, MW], F32, tag="wk")
                nc.vector.tensor_scalar(out=e4, in0=affw_t,
                                        scalar1=qaff_t[:, m:m + 1],
                                        scalar2=-POSC,
                                        op0=ALU.is_equal, op1=ALU.mult)
                # v4 = +POSC * (key_cid == query_cid)
                v4 = wk.tile([128, p2w], F32, tag="wk")
                nc.vector.tensor_scalar(out=v4, in0=cidw_t,
                                        scalar1=qcid_t[:, m:m + 1],
                                        scalar2=POSC, op0=ALU.is_equal,
                                        op1=ALU.mult)

                acc = small.tile([128, NGRP + 1], F32, tag="acc")
                s1 = wk.tile([128, MW], F32, tag="wk")  # sim - POSC*aff_eq

                for g in range(NGRP):
                    ps = psum.tile([128, GW], F32, tag="ps")
                    for k in range(2):
                        lhsT = qtt[k][:, m * 128:(m + 1) * 128]
                        for j in range(GW // 512):
                            nc.tensor.matmul(
                                ps[:, j * 512:(j + 1) * 512],
                                lhsT,
                                ktt[k][g][:, j * 512:(j + 1) * 512],
                                start=(k == 0), stop=(k == 1))
                    if g * GW < MW:
                        # masked sims double as the pass-2 input
                        sl = s1[:, g * GW:(g + 1) * GW]
                        nc.vector.tensor_add(sl, ps,
                                             e4[:, g * GW:(g + 1) * GW])
                        nc.vector.reduce_max(acc[:, g:g + 1], sl,
                                             axis=mybir.AxisListType.X)
                    else:
                        nc.vector.reduce_max(acc[:, g:g + 1], ps,
                                             axis=mybir.AxisListType.X)

                # hard_neg = max over group accs; b = hn + (MARGIN - POSC)
                nc.vector.tensor_reduce(acc[:, NGRP:NGRP + 1], acc[:, 0:NGRP],
                                        axis=mybir.AxisListType.X, op=ALU.max)
                bt = small.tile([128, 1], F32, tag="bt")
                nc.vector.tensor_scalar(out=bt, in0=acc[:, NGRP:NGRP + 1],
                                        scalar1=MARGIN - POSC, scalar2=None,
                                        op0=ALU.add)
                # s2 = sim + POSC*(cid_eq - aff_eq)
                s2 = wk.tile([128, p2w], F32, tag="wk")
                nc.gpsimd.tensor_add(s2, s1[:, 0:p2w], v4)
                # loss row-sum: relu(b - s2), accumulated over the window
                ldump = dmp.tile([128, p2w], F32, tag="dmp")
                nc.scalar.activation(ldump, s2, ACTF.Relu,
                                     bias=bt[:, 0:1], scale=-1.0,
                                     accum_out=lsum[:, m:m + 1])

            nc.sync.dma_start(out=out[:, :], in_=lsum)

    nc.finalize()
    return nc


def _prep(projections, affordance_ids, instance_ids, p2w):
    P = np.ascontiguousarray(np.asarray(projections, dtype=np.float32))
    aff = np.asarray(affordance_ids).astype(np.int64)
    inst = np.asarray(instance_ids).astype(np.int64)

    order = np.argsort(aff, kind="stable")
    P_s = P[order]
    aff_s = aff[order]
    inst_s = inst[order]
    imax = int(inst_s.max()) + 1
    cid_s = aff_s * imax + inst_s
    assert cid_s.max() < (1 << 24)

    amax = int(aff_s.max()) + 1
    gstart = np.searchsorted(aff_s, np.arange(amax), side="left")
    gend = np.searchsorted(aff_s, np.arange(amax), side="right")

    in_maps = []
    for c in range(NCORES):
        r0, r1 = c * RPC, (c + 1) * RPC
        S_c = int(gstart[aff_s[r0]])
        E_c = int(gend[aff_s[r1 - 1]])
        w_c = E_c - S_c
        if w_c > p2w:
            return None  # caller rebuilds with a wider window
        key_order = np.concatenate([
            np.arange(S_c, E_c), np.arange(0, S_c), np.arange(E_c, B)])

        kt_np = np.ascontiguousarray(P_s[key_order].T)
        qt_np = np.ascontiguousarray(P_s[r0:r1].T)
        # key affs over the masked region; cids over the pass-2 window
        affk = aff_s[key_order[:MW]].astype(np.float32)
        affk[w_c:] = -1.0  # outside the window: never equal to a query aff
        affw_np = np.ascontiguousarray(np.broadcast_to(affk[None, :],
                                                       (128, MW)))
        cidw_np = np.ascontiguousarray(np.broadcast_to(
            cid_s[key_order[:p2w]].astype(np.float32)[None, :], (128, p2w)))

        def tile_fmt(x):
            return np.ascontiguousarray(
                x.astype(np.float32).reshape(NT, 128).T)

        in_maps.append({
            "kt": kt_np, "qt": qt_np, "affw": affw_np, "cidw": cidw_np,
            "qaff": tile_fmt(aff_s[r0:r1].astype(np.float32)),
            "qcid": tile_fmt(cid_s[r0:r1].astype(np.float32)),
        })

    # num_pairs / valid rows are pure id metadata
    gsize = (gend - gstart).astype(np.int64)
    cid_u, cid_cnt = np.unique(cid_s, return_counts=True)
    cnt_map = dict(zip(cid_u.tolist(), cid_cnt.tolist()))
    ccnt = np.fromiter((cnt_map[c] for c in cid_s.tolist()), dtype=np.int64,
                       count=B)
    poscnt = gsize[aff_s] - ccnt
    negcnt = B - gsize[aff_s]
    valid = (poscnt > 0) & (negcnt > 0)
    num_pairs = int(poscnt[valid].sum())
    return in_maps, num_pairs


def kernel(projections, affordance_ids, instance_ids):
    for p2w in (1536, 2048):
        prep = _prep(projections, affordance_ids, instance_ids, p2w)
        if prep is not None:
            break
    else:
        raise AssertionError("positive window exceeds 2048 columns")
    in_maps, num_pairs = prep
    if p2w not in _cache:
        _cache[p2w] = build_kernel(p2w)
    nc = _cache[p2w]
    res = bass_utils.run_bass_kernel_spmd(nc, in_maps,
                                          core_ids=list(range(NCORES)))
    total = 0.0
    for c in range(NCORES):
        total += res.results[c]["out"].astype(np.float64).sum()
    if num_pairs > 0:
        val = np.float32(np.float32(total) / np.float32(num_pairs))
    else:
        val = np.float32(0.0)
    return np.asarray(val, dtype=np.float32)


# revision 20
# speedup vs baseline: 9.9947x; 9.9947x over previous
"""AdversarialContrastiveLoss on 8 trn2 NeuronCores.

Strategy (per sharding hint): shard rows of the 8192x8192 similarity matrix
across 8 cores (1024 query rows each); every core holds all 8192 keys.

Host-side prep (index metadata only — all O(B^2 D) work stays on device):
  * rows sorted by affordance id; a per-core key rotation puts every
    own-affordance key of the core's queries in columns [0, 2048)
  * projections pre-transposed to [D, B] so the PE gets contraction on
    partitions without on-device transposes
  * one-hot affordance codes appended to the contraction dim: the PE's
    third accumulation pass adds -4*(aff_q == aff_k) straight into PSUM
    (sims are in [-1,1], so -4 acts as -inf for the row max, and exactly
    cancels against the +4 in the relu bias for positive pairs)

Device (per core, single fused pass over sim):
  * PSUM tile [128 q x 2048 k] via float32r matmuls (N=512): 2 K=128
    passes of projections + 1 K=36 one-hot mask pass (window tile only)
  * hard-negative row max: one VectorE reduce_max per PSUM tile
  * margin-loss row sums: one ScalarE relu(bias - x) with accumulate,
    reading the masked window PSUM tile directly
  * same-cid (aff-equal, instance-equal) pairs are NOT excluded on device;
    the host subtracts those few terms (~0.01% of pairs) using the
    device-exported hard-negative values — pure id metadata + a handful of
    dot products
  * outputs per-partition loss sums + per-row hard negatives
"""

import os
import sys

try:
    import concourse  # noqa: F401  (resolves via the container's sitecustomize)
except ImportError:  # pragma: no cover - fallback for bare environments
    for _p in ("/root/.axon_site/_ro/trn_rl_repo", "/opt/trn_rl_repo"):
        if os.path.isdir(_p) and _p not in sys.path:
            sys.path.append(_p)

import numpy as np

import concourse.bass as bass
import concourse.tile as tile
from concourse import bacc, bass_utils, mybir

F32 = mybir.dt.float32
F32R = mybir.dt.float32r  # TF32-like PE mode: 4x matmul throughput vs fp32
ALU = mybir.AluOpType
ACTF = mybir.ActivationFunctionType
MM_DT = F32R

B = 8192
D = 256
NCORES = 8
RPC = B // NCORES            # query rows per core
NT = RPC // 128              # query tiles per core (8)
GW = 2048                    # cols per PSUM tile (4 banks)
NGRP = B // GW               # 4 psum groups
NAFF = 64                    # one-hot rows (>= #affordance classes, padded)
MARGIN = 0.2
POSC = 4.0                   # mask offset: exact, and > max margin excess
_cache = {}


def build_kernel():
    nc = bacc.Bacc("TRN2", target_bir_lowering=False)

    kt = nc.dram_tensor("kt", [D, B], F32, kind="ExternalInput")
    qt = nc.dram_tensor("qt", [D, RPC], F32, kind="ExternalInput")
    kh = nc.dram_tensor("kh", [NAFF, GW], F32, kind="ExternalInput")
    qh = nc.dram_tensor("qh", [NAFF, RPC], F32, kind="ExternalInput")
    out = nc.dram_tensor("out", [128, NT], F32, kind="ExternalOutput")
    hno = nc.dram_tensor("hno", [128, NT], F32, kind="ExternalOutput")

    with tile.TileContext(nc) as tc:
        with tc.tile_pool(name="singles", bufs=1) as singles, \
             tc.tile_pool(name="dmp", bufs=2) as dmp, \
             tc.tile_pool(name="small", bufs=4) as small, \
             tc.tile_pool(name="psw", bufs=1, space="PSUM") as psw, \
             tc.tile_pool(name="psum", bufs=2, space="PSUM") as psum:

            # queries first (every matmul needs them), then keys in group
            # order so compute can start as groups land
            qtt = [singles.tile([128, RPC], MM_DT, tag=f"qt{k}",
                                name=f"qtt{k}")
                   for k in range(2)]
            for k in range(2):
                nc.sync.dma_start(out=qtt[k],
                                  in_=qt[k * 128:(k + 1) * 128, :]
                                  .bitcast(MM_DT))
            qh_t = singles.tile([NAFF, RPC], MM_DT, tag="qh")
            nc.scalar.dma_start(out=qh_t, in_=qh[:, :].bitcast(MM_DT))
            kh_t = singles.tile([NAFF, GW], MM_DT, tag="kh")
            nc.scalar.dma_start(out=kh_t, in_=kh[:, :].bitcast(MM_DT))

            ktt = [[singles.tile([128, GW], MM_DT, tag=f"kt{k}g{g}",
                                 name=f"ktt{k}g{g}")
                    for g in range(NGRP)] for k in range(2)]
            dma_engines = [nc.sync, nc.scalar]
            di = 0
            for g in range(NGRP):
                for k in range(2):
                    for h in range(2):  # split chunks across both queues
                        dma_engines[di % 2].dma_start(
                            out=ktt[k][g][:, h * (GW // 2):(h + 1) * (GW // 2)],
                            in_=kt[k * 128:(k + 1) * 128,
                                   g * GW + h * (GW // 2):
                                   g * GW + (h + 1) * (GW // 2)]
                            .bitcast(MM_DT))
                        di += 1

            lsum = singles.tile([128, NT], F32, tag="lsum")
            hnt = singles.tile([128, NT], F32, tag="hnt")

            NSUB = (B - GW) // 1024  # non-window sub-groups of 1024 cols

            for m in range(NT):
                acc = small.tile([128, NSUB + 2], F32, tag="acc")
                # window tile [128, 2048]: 2 K=128 passes + 1 one-hot pass
                ps0 = psw.tile([128, GW], F32, tag="psw")
                for k in range(2):
                    lhsT = qtt[k][:, m * 128:(m + 1) * 128]
                    for j in range(GW // 512):
                        nc.tensor.matmul(
                            ps0[:, j * 512:(j + 1) * 512], lhsT,
                            ktt[k][0][:, j * 512:(j + 1) * 512],
                            start=(k == 0), stop=False)
                lhsT = qh_t[:, m * 128:(m + 1) * 128]
                for j in range(GW // 512):
                    nc.tensor.matmul(
                        ps0[:, j * 512:(j + 1) * 512], lhsT,
                        kh_t[:, j * 512:(j + 1) * 512],
                        start=False, stop=True)
                nc.vector.reduce_max(acc[:, 0:1], ps0,
                                     axis=mybir.AxisListType.X)

                # remaining cols in [128, 1024] sub-groups (2 K-passes)
                for sg in range(NSUB):
                    g = 1 + sg // 2
                    lo = (sg % 2) * 1024
                    ps = psum.tile([128, 1024], F32, tag="ps")
                    for k in range(2):
                        lhsT = qtt[k][:, m * 128:(m + 1) * 128]
                        for j in range(2):
                            nc.tensor.matmul(
                                ps[:, j * 512:(j + 1) * 512], lhsT,
                                ktt[k][g][:, lo + j * 512:lo + (j + 1) * 512],
                                start=(k == 0), stop=(k == 1))
                    nc.vector.reduce_max(acc[:, sg + 1:sg + 2], ps,
                                         axis=mybir.AxisListType.X)

                # hard_neg; relu bias b = hn + (MARGIN - POSC)
                nc.vector.tensor_reduce(hnt[:, m:m + 1], acc[:, 0:NSUB + 1],
                                        axis=mybir.AxisListType.X, op=ALU.max)
                bt = small.tile([128, 1], F32, tag="bt")
                nc.vector.tensor_scalar(out=bt, in0=hnt[:, m:m + 1],
                                        scalar1=MARGIN - POSC, scalar2=None,
                                        op0=ALU.add)
                # loss row-sum over the window tile, straight from PSUM
                ldump = dmp.tile([128, GW], F32, tag="dmp")
                nc.scalar.activation(ldump, ps0, ACTF.Relu,
                                     bias=bt[:, 0:1], scale=-1.0,
                                     accum_out=lsum[:, m:m + 1])

            nc.sync.dma_start(out=out[:, :], in_=lsum)
            nc.sync.dma_start(out=hno[:, :], in_=hnt)

    nc.finalize()
    return nc


def _prep(projections, affordance_ids, instance_ids):
    P = np.ascontiguousarray(np.asarray(projections, dtype=np.float32))
    aff = np.asarray(affordance_ids).astype(np.int64)
    inst = np.asarray(instance_ids).astype(np.int64)

    order = np.argsort(aff, kind="stable")
    P_s = P[order]
    aff_s = aff[order]
    inst_s = inst[order]
    imax = int(inst_s.max()) + 1
    cid_s = aff_s * imax + inst_s

    amax = int(aff_s.max()) + 1
    assert amax <= NAFF
    gstart = np.searchsorted(aff_s, np.arange(amax), side="left")
    gend = np.searchsorted(aff_s, np.arange(amax), side="right")

    in_maps = []
    meta = []
    for c in range(NCORES):
        r0, r1 = c * RPC, (c + 1) * RPC
        S_c = int(gstart[aff_s[r0]])
        E_c = int(gend[aff_s[r1 - 1]])
        w_c = E_c - S_c
        assert w_c <= GW, f"core {c}: own-aff window {w_c} > {GW}"
        key_order = np.concatenate([
            np.arange(S_c, E_c), np.arange(0, S_c), np.arange(E_c, B)])

        kt_np = np.ascontiguousarray(P_s[key_order].T)
        qt_np = np.ascontiguousarray(P_s[r0:r1].T)
        # one-hot affordance codes for the PE mask pass
        kh_np = np.zeros((NAFF, GW), dtype=np.float32)
        kw = key_order[:GW]
        kh_np[aff_s[kw], np.arange(GW)] = 1.0
        qh_np = np.zeros((NAFF, RPC), dtype=np.float32)
        qh_np[aff_s[r0:r1], np.arange(RPC)] = -POSC

        in_maps.append({"kt": kt_np, "qt": qt_np, "kh": kh_np, "qh": qh_np})
        meta.append((r0, r1))

    # --- id metadata: num_pairs + the same-cid pair list -------------------
    gsize = (gend - gstart).astype(np.int64)
    cid_u, inv, cid_cnt = np.unique(cid_s, return_inverse=True,
                                    return_counts=True)
    ccnt = cid_cnt[inv]
    poscnt = gsize[aff_s] - ccnt
    negcnt = B - gsize[aff_s]
    valid = (poscnt > 0) & (negcnt > 0)
    num_pairs = int(poscnt[valid].sum())

    # pairs (q, k) with equal cid (includes q == k). The device's loss sum
    # includes relu(hn_q + MARGIN - sim_qk) for them; subtract on host.
    ord2 = np.argsort(inv, kind="stable")
    cid_sorted = inv[ord2]
    runs = np.searchsorted(cid_sorted, np.arange(len(cid_u) + 1))
    pair_q, pair_k = [], []
    for u in range(len(cid_u)):
        lo, hi = runs[u], runs[u + 1]
        members = ord2[lo:hi]
        for i in members:
            for j in members:
                pair_q.append(i)
                pair_k.append(j)
    pair_q = np.asarray(pair_q, dtype=np.int64)
    pair_k = np.asarray(pair_k, dtype=np.int64)

    return in_maps, num_pairs, (P_s, pair_q, pair_k)


def kernel(projections, affordance_ids, instance_ids):
    in_maps, num_pairs, (P_s, pair_q, pair_k) = _prep(
        projections, affordance_ids, instance_ids)
    if "nc" not in _cache:
        _cache["nc"] = build_kernel()
    nc = _cache["nc"]
    res = bass_utils.run_bass_kernel_spmd(nc, in_maps,
                                          core_ids=list(range(NCORES)))
    total = 0.0
    hn = np.empty(B, dtype=np.float32)
    for c in range(NCORES):
        total += res.results[c]["out"].astype(np.float64).sum()
        # hno[:, m] holds rows c*RPC + m*128 ... + 128
        hn[c * RPC:(c + 1) * RPC] = res.results[c]["hno"].T.reshape(-1)

    # host correction: remove same-cid (incl. self) pair contributions
    sims = np.einsum("ij,ij->i", P_s[pair_q], P_s[pair_k]).astype(np.float32)
    b = (hn[pair_q] + np.float32(MARGIN - POSC)).astype(np.float32)
    corr = np.maximum(b - (sims - np.float32(POSC)), np.float32(0.0))
    total -= corr.astype(np.float64).sum()

    if num_pairs > 0:
        val = np.float32(np.float32(total) / np.float32(num_pairs))
    else:
        val = np.float32(0.0)
    return np.asarray(val, dtype=np.float32)
